# revision 52
# baseline (speedup 1.0000x reference)
"""Trainium2 Bass kernel for nn_EncoderLayer (multiplicative-attention encoder layer).

Sharding: 8 cores; core c handles batch b=c//2, head-group hg=c%2 (4 of 8 heads).
The reference's head-major reshape bug maps head h to output rows [256h, 256h+256),
so each core owns 1024 complete output rows -> no collectives.

v2 design (fp8 DoubleRow everywhere except the FFN matmuls):
  - Q/K/V projections: fp8-DR (contraction 2x128 per instr), evictions add bias
    and rescale on DVE, outputs fp8.
  - energy: e[k,q] = k2.T @ q2 with d split in 2x32 fp8-DR subtiles; q2/k2 built
    by SBUF->SBUF DMA rearrange from the eviction staging tiles.
  - exp on ACT (bias = mask bias, scale=1/4) -> at2 fp8 tiles [128, 2(kt), 1024q].
  - AV query-major: x'[q,d] += at2(kt-pair).T @ V via fp8-DR, M=128 queries/instr;
    denominator via ones-column of V into a tiny psum tile.
  - normalize (x64) on DVE -> bf16, PE-transpose back to [d, q], copies build the
    fp8 xnp (straight + shifted plane for the K=128 Wo operand trick).
  - Wo fp8-DR over j-pairs; FFN bf16; LN stats via fp8-DR ones-matmuls;
    LN rsqrt = exp(-0.5*ln(v+eps)) on ACT.
  - chain (Wo+LN1+FFN+LN2+out) is split per (pair, head) into 256-col halves and
    interleaved into the ACT-bound attention span via a unit queue.
Host reassembles out[b, 1024*hg : 1024*(hg+1), :] = out_t.T per core.
"""

import os as _os_env
# The shared trn2 device occasionally stays wedged after another process
# crashes; a core reset at runtime init recovers it and is harmless otherwise.
_os_env.environ.setdefault("NEURON_RT_RESET_CORES", "1")

import numpy as np
import ml_dtypes

import concourse.bass as bass
import concourse.tile as tile
import concourse.bacc as bacc
from concourse import mybir
from concourse import bass_utils
from concourse import hw_specs as _hw_specs

_real_gat = _hw_specs.get_activation_tables


def _gat_pinned(arch):
    tabs = _real_gat(arch)
    return {name: (fns if name == "natural_log_exp_and_others" else set())
            for name, fns in tabs.items()}


bacc.get_activation_tables = _gat_pinned

B, S, HID, H, PF, D = 4, 2048, 512, 8, 2048, 64
N_CORES = 8
HPC = H // 2          # heads per core (4)
R = HPC * 256         # output rows per core (1024)
F32 = mybir.dt.float32
BF16 = mybir.dt.bfloat16
F8 = mybir.dt.float8e4
AF = mybir.ActivationFunctionType
OP = mybir.AluOpType
DR = mybir.MatmulPerfMode.DoubleRow
NEG_BIG = -87.0
LN_EPS = 1e-5

SWQ = 64.0    # wq scale (wq values tiny)
SW = 16.0     # wk/wv/wo scale
SQ = 4.0      # q_sb holds 4*Qm
SX = 64.0     # xnp holds 64*x

_built_cache = {}
last_results = None
run_kwargs = {}

f8dt = (ml_dtypes.float8_e4m3fn if hasattr(ml_dtypes, 'float8_e4m3fn')
        else ml_dtypes.float8_e4m3)
bfdt = ml_dtypes.bfloat16


def _bcast_ap(ap_1d, parts):
    return bass.AP(tensor=ap_1d.tensor, offset=ap_1d.offset,
                   ap=[[0, parts], *ap_1d.ap])


def build_bass(sup):
    KT = sup // 128
    NFULL = KT // 2       # full DR kt-pairs
    ODD = KT % 2
    NT = NFULL + ODD      # at2 tiles per (head, q0)
    nc = bacc.Bacc("TRN2", target_bir_lowering=False, debug=False,
                   num_devices=N_CORES)

    def inp(name, shape, dt=F32):
        return nc.dram_tensor(name, shape, dt, kind="ExternalInput").ap()

    src8_d = inp("src8", [128, 4 * S], F8)
    srcu8_d = inp("srcu8", [128, 4 * sup], F8)
    src_res_d = inp("src_res", [HID, R])        # src[b].T slice + bo
    wq8_d = inp("wq8", [128, 4 * 256], F8)
    wk8_d = inp("wk8", [128, 4 * 256], F8)
    wv8_d = inp("wv8", [128, 4 * 256], F8)
    wo8_d = inp("wo8", [128, 4 * 512], F8)
    w1_d = inp("w1", [HID, PF], BF16)
    w2_d = inp("w2", [PF, HID], BF16)
    bq_d = inp("bq", [2, 128])                  # 64*(Wm bq + bm)
    bk_d = inp("bk", [2, 128])                  # 16*bk
    bv_d = inp("bv", [256])                     # bv (free-dim bias)
    b1_d = inp("b1", [16, 128])
    b2_d = inp("b2", [4, 128])
    g1_d = inp("g1", [4, 128])
    bt1_d = inp("bt1", [4, 128])
    g2_d = inp("g2", [4, 128])
    bt2_d = inp("bt2", [4, 128])
    mb_d = inp("mb", [KT, 128])
    id_d = inp("ident", [128, 128], BF16)
    out_d = nc.dram_tensor("out_t", [HID, R], F32, kind="ExternalOutput").ap()

    from contextlib import ExitStack
    with tile.TileContext(nc) as tc, ExitStack() as ctx:
        con = ctx.enter_context(tc.tile_pool(name="con", bufs=1))
        ppe = ctx.enter_context(tc.tile_pool(name="ppe", bufs=2, space="PSUM"))
        pav = ctx.enter_context(tc.tile_pool(name="pav", bufs=1, space="PSUM"))
        paux = ctx.enter_context(tc.tile_pool(name="paux", bufs=1, space="PSUM"))
        pch = ctx.enter_context(tc.tile_pool(name="pch", bufs=2, space="PSUM"))
        at_pool = ctx.enter_context(tc.tile_pool(name="at", bufs=3))
        xq_pool = ctx.enter_context(tc.tile_pool(name="xq", bufs=2))
        rep_pool = ctx.enter_context(tc.tile_pool(name="rep", bufs=2))
        z_pool = ctx.enter_context(tc.tile_pool(name="z", bufs=2))
        zb_pool = ctx.enter_context(tc.tile_pool(name="zb", bufs=2))
        h1_pool = ctx.enter_context(tc.tile_pool(name="h1", bufs=2))
        tmp_pool = ctx.enter_context(tc.tile_pool(name="tmp", bufs=2))
        o_pool = ctx.enter_context(tc.tile_pool(name="o", bufs=4))

        mm = nc.tensor.matmul
        act = nc.scalar.activation
        dve = nc.vector
        gps = nc.gpsimd

        def dma(out, in_):
            nc.sync.dma_start(out=out, in_=in_)

        def ctile(shape, dt, tag):
            return con.tile(shape, dt, tag=tag, name=tag)

        # ---- constant tiles ----
        srcu8 = ctile([128, max(4 * sup, 4 * R)], F8, "srcu8")
        wk8 = ctile([128, 4 * 256], F8, "wk8")
        wv8 = ctile([128, 4 * 256], F8, "wv8")
        dma(wk8, wk8_d)
        dma(srcu8[:, :4 * sup], srcu8_d)
        src8 = ctile([128, 4 * S], F8, "src8")
        wq8 = ctile([128, 4 * 256], F8, "wq8")
        dma(wv8, wv8_d)
        dma(src8, src8_d)
        dma(wq8, wq8_d)
        wo8 = ctile([128, 4 * 512], F8, "wo8")
        w1_sb = [ctile([128, PF], BF16, f"w1{i}") for i in range(4)]
        w2_sb = [ctile([128, 512], BF16, f"w2{i}") for i in range(16)]
        src_res = [ctile([128, R], F32, f"srcres{i}") for i in range(4)]

        def load_chain_weights():
            dma(wo8, wo8_d)
            for i in range(4):
                dma(w1_sb[i], w1_d[128 * i:128 * (i + 1), :])
            for i in range(16):
                dma(w2_sb[i], w2_d[128 * i:128 * (i + 1), :])
            for i in range(4):
                dma(src_res[i], src_res_d[128 * i:128 * (i + 1), :])

        def vec_in(dram, n, tag):
            t = ctile([128, n], F32, tag)
            dma(t, dram.rearrange("m p -> p m"))
            return t

        bq_sb = vec_in(bq_d, 2, "bq")
        bk_sb = vec_in(bk_d, 2, "bk")
        b1_sb = vec_in(b1_d, 16, "b1")
        b2_sb = vec_in(b2_d, 4, "b2")
        g1_sb = vec_in(g1_d, 4, "g1")
        bt1_sb = vec_in(bt1_d, 4, "bt1")
        g2_sb = vec_in(g2_d, 4, "g2")
        bt2_sb = vec_in(bt2_d, 4, "bt2")
        mb_sb = vec_in(mb_d, KT, "mb")
        bv_rep = ctile([128, 256], F32, "bvrep")
        dma(bv_rep, _bcast_ap(bv_d, 128))
        ident = ctile([128, 128], BF16, "ident")
        dma(ident, id_d)

        import os as _os
        ST8 = int(_os.environ.get("KF_ST8", "1"))
        DRVFINE = int(_os.environ.get("KDRVF", "0"))
        TBD = int(_os.environ.get("KTBD", "0"))
        CHD = int(_os.environ.get("KCHD", "6"))
        PRI = int(_os.environ.get("KPRI", "0"))
        LASTD = int(_os.environ.get("KLASTD", "1"))
        ones8 = ctile([128, 2 * 128], F8 if ST8 else BF16, "ones8")
        dve.memset(ones8, 1.0)
        eps_t = ctile([128, 1], F32, "eps")
        dve.memset(eps_t, LN_EPS)

        src8v = src8.rearrange("p (c s q) -> p c s q", c=2, s=2)
        srcu8v = srcu8[:, :4 * sup].rearrange("p (c s q) -> p c s q", c=2, s=2)
        wq8v = wq8.rearrange("p (c s m) -> p c s m", c=2, s=2)
        wk8v = wk8.rearrange("p (c s m) -> p c s m", c=2, s=2)
        wv8v = wv8.rearrange("p (c s m) -> p c s m", c=2, s=2)
        wo8v = wo8.rearrange("p (g s m) -> p g s m", g=2, s=2)

        # ---- projections (fp8-DR) ----
        q_sb = [ctile([128, S], F8, f"q{g}") for g in range(2)]
        k_sb = [ctile([128, sup], F8, f"k{g}") for g in range(2)]

        def proj(w8v, src_v, n_total, out_sb, g, bias_sb, rescale):
            n0 = 0
            while n0 < n_total:
                nq = min(512, n_total - n0)
                ps = pch.tile([128, 512], F32, tag="ps", name="ps")
                for c in range(2):
                    mm(ps[:, :nq], w8v[:, c, :, 128 * g:128 * (g + 1)],
                       src_v[:, c, :, n0:n0 + nq],
                       start=(c == 0), stop=(c == 1), perf_mode=DR)
                dve.tensor_scalar(out=out_sb[:, n0:n0 + nq], in0=ps[:, :nq],
                                  scalar1=bias_sb[:, g:g + 1], scalar2=rescale,
                                  op0=OP.add, op1=OP.mult)
                n0 += nq

        # V natural [keys, 4 heads x (64 | ones)] with 65-stride
        v_sb = ctile([128, KT * 4 * 65], F8, "v")
        v_v = v_sb.rearrange("p (kt h e) -> p kt h e", kt=KT, h=4)
        dve.memset(v_v[:, :, :, 64], 1.0 / SX)   # den scaled so recip = SX/den

        def proj_v():
            for kt in range(KT):
                ps = pch.tile([128, 512], F32, tag="ps", name="ps")
                for c in range(2):
                    mm(ps[:, :256], srcu8v[:, c, :, 128 * kt:128 * (kt + 1)],
                       wv8v[:, c, :, :], start=(c == 0), stop=(c == 1),
                       perf_mode=DR)
                dve.scalar_tensor_tensor(
                    out=v_v[:, kt, :, 0:64],
                    in0=ps[:, :256].rearrange("p (h d) -> p h d", h=4),
                    scalar=1.0 / SW, in1=bv_rep.rearrange("p (h d) -> p h d", h=4),
                    op0=OP.mult, op1=OP.add)

        # d-split rearranged copies for the energy matmul
        q2 = [ctile([32, 2 * S], F8, f"q2_{h}") for h in range(4)]
        k2 = [ctile([32, 2 * sup], F8, f"k2_{h}") for h in range(4)]

        def rearrange_qk(h):
            g, p0 = h // 2, 64 * (h % 2)
            dma(q2[h][:, 0:S], q_sb[g][p0:p0 + 32, :])
            dma(q2[h][:, S:2 * S], q_sb[g][p0 + 32:p0 + 64, :])
            dma(k2[h][:, 0:sup], k_sb[g][p0:p0 + 32, :])
            dma(k2[h][:, sup:2 * sup], k_sb[g][p0 + 32:p0 + 64, :])

        q2v = [q2[h].rearrange("p (s q) -> p s q", s=2) for h in range(4)]
        k2v = [k2[h].rearrange("p (s q) -> p s q", s=2) for h in range(4)]

        # ---- chain unit queue (interleaved into attention) ----
        from collections import deque
        units = deque()

        def drive(n):
            for _ in range(n):
                if units:
                    units.popleft()()

        def drain_units():
            while units:
                units.popleft()()

        xnp = [ctile([128, 2 * S], F8, f"xnp{g}") for g in range(2)]

        # ---- attention for head h ----
        def av_mms(h, at_tiles, xp_v, cq0):
            # xp_v: [128, j(4), e(65)] psum view; cq0: at2 col offset (0 or 512)
            # Only the FIRST mm of the tile uses start=True: a start re-arms the
            # whole 2KB zero region, so per-j starts would discard sibling
            # blocks' earlier accumulation. The armed bank zero-fills each
            # block's first write instead.
            for t in range(NT):
                solo = (t == NFULL)
                at2v = at_tiles[t]
                for j in range(4):
                    qs = slice(cq0 + 128 * j, cq0 + 128 * (j + 1))
                    st = (t == 0) and (j == 0)
                    if solo:
                        mm(xp_v[:, j, :], at2v[:, 0, qs],
                           v_v[:, KT - 1, h, :], start=st, stop=True,
                           skip_group_check=True)
                    else:
                        mm(xp_v[:, j, :], at2v[:, :, qs],
                           v_v[:, 2 * t:2 * t + 2, h, :],
                           start=st, stop=(t == NT - 1), perf_mode=DR,
                           skip_group_check=True)

        def norm_transpose(xnp_g, xp_v, c0, dn):
            rep = rep_pool.tile([128, 4], F32, tag="rep", name="rep")
            dve.reciprocal(rep, xp_v[:, :, 64])
            xq = xq_pool.tile([128, 256], BF16, tag="xq", name="xq")
            for j in range(4):
                dve.tensor_scalar_mul(xq[:, 64 * j:64 * (j + 1)],
                                      xp_v[:, j, 0:64], rep[:, j:j + 1])
            tp = paux.tile([64, 512], BF16, tag="tp", name="tp")
            for j in range(4):
                nc.tensor.transpose(tp[:, 128 * j:128 * (j + 1)],
                                    xq[:, 64 * j:64 * (j + 1)], ident)
            dve.tensor_copy(out=xnp_g[0:64, c0:c0 + 512], in_=tp)
            # shifted plane from the straight plane (gpsimd cannot read PSUM)
            if c0 % S == 0:
                gps.tensor_copy(out=xnp_g[64:128, c0:c0 + 511],
                                in_=xnp_g[0:64, c0 + 1:c0 + 512])
            else:
                gps.tensor_copy(out=xnp_g[64:128, c0 - 1:c0 + 511],
                                in_=xnp_g[0:64, c0:c0 + 512])
            drive(dn * (1 if DRVFINE else CHD))

        def attention(h, dn, prefetch=None):
            g = h // 2
            o0 = S * (h % 2)
            xnp_g = xnp[g]
            for q0 in (0, S // 2):
                at_tiles = []
                xpA = pav.tile([128, 260], F32, tag="xp", name="xp")
                xpA_v = xpA.rearrange("p (j e) -> p j e", j=4)
                for t in range(NT):
                    solo = (t == NFULL)
                    at2 = at_pool.tile([128, 2 * 1024], F8, tag="at2",
                                       name="at2", bufs=NT + 1)
                    at2v = at2.rearrange("p (i q) -> p i q", i=2)
                    at_tiles.append(at2v)
                    for i in range(1 if solo else 2):
                        kt = 2 * t + i
                        e = ppe.tile([128, 1024], F32, tag="e", name="e")
                        for half in range(2):
                            mm(e[:, 512 * half:512 * (half + 1)],
                               k2v[h][:, :, 128 * kt:128 * (kt + 1)],
                               q2v[h][:, :, q0 + 512 * half:q0 + 512 * (half + 1)],
                               start=True, stop=True, perf_mode=DR,
                               skip_group_check=True)
                        act(at2v[:, i, :], e, AF.Exp,
                            bias=mb_sb[:, kt:kt + 1], scale=1.0 / SQ)
                    for j in range(4):
                        qs = slice(128 * j, 128 * (j + 1))
                        st = (t == 0) and (j == 0)
                        if solo:
                            mm(xpA_v[:, j, :], at2v[:, 0, qs],
                               v_v[:, KT - 1, h, :], start=st, stop=True,
                               skip_group_check=True)
                        else:
                            mm(xpA_v[:, j, :], at2v[:, :, qs],
                               v_v[:, 2 * t:2 * t + 2, h, :],
                               start=st, stop=(t == NT - 1), perf_mode=DR,
                               skip_group_check=True)
                    drive(dn * TBD)
                norm_transpose(xnp_g, xpA_v, o0 + q0, dn)
                xpB = pav.tile([128, 260], F32, tag="xp", name="xp")
                xpB_v = xpB.rearrange("p (j e) -> p j e", j=4)
                av_mms(h, at_tiles, xpB_v, 512)
                norm_transpose(xnp_g, xpB_v, o0 + q0 + 512, dn)

        # ---- layernorm on a 256-col block: z_tiles 4x[128,256] f32 ----
        def layernorm(z_tiles, g_sb, b_sb, writers, W=256):
            s1 = pch.tile([128, W], F32, tag="ps", name="ps")
            s2 = pch.tile([128, W], F32, tag="ps", name="ps")
            o8 = ones8.rearrange("p (i m) -> p i m", i=2)
            for cp in range(2):
                zdt = F8 if ST8 else BF16
                zt = zb_pool.tile([128, 2 * W], zdt, tag=f"zp{cp}", name=f"zp{cp}")
                st = zb_pool.tile([128, 2 * W], zdt, tag=f"sp{cp}", name=f"sp{cp}")
                ztv = zt.rearrange("p (i w) -> p i w", i=2)
                stv = st.rearrange("p (i w) -> p i w", i=2)
                for i in range(2):
                    ct = 2 * cp + i
                    gps.tensor_copy(out=ztv[:, i, :], in_=z_tiles[ct])
                    gps.tensor_mul(stv[:, i, :], z_tiles[ct], z_tiles[ct])
                if ST8:
                    mm(s1, o8, ztv, start=(cp == 0), stop=(cp == 1),
                       perf_mode=DR, skip_group_check=True)
                    mm(s2, o8, stv, start=(cp == 0), stop=(cp == 1),
                       perf_mode=DR, skip_group_check=True)
                else:
                    for i in range(2):
                        mm(s1, o8[:, i, :], ztv[:, i, :],
                           start=(cp == 0 and i == 0), stop=(cp == 1 and i == 1),
                           skip_group_check=True)
                        mm(s2, o8[:, i, :], stv[:, i, :],
                           start=(cp == 0 and i == 0), stop=(cp == 1 and i == 1),
                           skip_group_check=True)
            bm = tmp_pool.tile([128, W], F32, tag="bm", name="bm")
            br = tmp_pool.tile([128, W], F32, tag="br", name="br")
            m2 = tmp_pool.tile([128, W], F32, tag="m2", name="m2")
            dve.tensor_scalar_mul(bm, s1, 1.0 / HID)
            dve.tensor_mul(m2, bm, bm)
            dve.scalar_tensor_tensor(out=br, in0=s2, scalar=1.0 / HID, in1=m2,
                                     op0=OP.mult, op1=OP.subtract)
            act(br, br, AF.Ln, bias=eps_t)
            act(br, br, AF.Exp, scale=-0.5)
            for ct in range(4):
                sub = tmp_pool.tile([128, W], F32, tag="sub", name="sub")
                gps.tensor_tensor(out=sub, in0=z_tiles[ct], in1=bm,
                                  op=OP.subtract)
                t2 = tmp_pool.tile([128, W], F32, tag="t2", name="t2")
                gps.tensor_tensor(out=t2, in0=sub, in1=br, op=OP.mult)
                writers(ct, t2, g_sb, b_sb)   # writer applies *g + b

        # src1 (bf16) serves as fc1 input and LN2 residual
        src1_b = ctile([128, 4 * R], BF16, "src1b")
        src1_bv = src1_b.rearrange("p (c r) -> p c r", c=4)
        h1_tiles = {}

        # ---- chain for (pair hp, head-half hh): 256 output cols ----
        def chain_half(hp, hh):
            c0 = 512 * hp + 256 * hh
            xv = xnp[hp].rearrange("p (a m g s o) -> p g s o a m",
                                   a=2, m=256, g=2, s=2)
            z1 = [z_pool.tile([128, 256], F32, tag=f"z{mt}", name=f"z{mt}")
                  for mt in range(4)]

            def wo_unit(mt):
                ps = pch.tile([128, 512], F32, tag="ps", name="ps")
                for jg in range(2):
                    mm(ps[:, :256], wo8v[:, jg, :, 128 * mt:128 * (mt + 1)],
                       xv[:, jg, :, 0, hh, :], start=(jg == 0), stop=(jg == 1),
                       perf_mode=DR)
                dve.scalar_tensor_tensor(out=z1[mt], in0=ps[:, :256],
                                         scalar=1.0 / (SX * SW),
                                         in1=src_res[mt][:, c0:c0 + 256],
                                         op0=OP.mult, op1=OP.add)

            def w1(ct, t2, g_sb, b_sb):
                dve.tensor_scalar(out=src1_bv[:, ct, c0:c0 + 256], in0=t2,
                                  scalar1=g_sb[:, ct:ct + 1],
                                  scalar2=b_sb[:, ct:ct + 1],
                                  op0=OP.mult, op1=OP.add)

            for mt in range(4):
                units.append(lambda mt=mt: wo_unit(mt))
            units.append(lambda: layernorm(z1, g1_sb, bt1_sb, w1))

            h1 = h1_pool.tile([128, 16 * 256], BF16, tag="h1", name="h1")
            h1_tiles[(hp, hh)] = h1

            def fc1_unit(mt):
                ps = pch.tile([128, 512], F32, tag="ps", name="ps")
                for ct in range(4):
                    mm(ps[:, :256], w1_sb[ct][:, 128 * mt:128 * (mt + 1)],
                       src1_bv[:, ct, c0:c0 + 256], start=(ct == 0),
                       stop=(ct == 3))
                if hp == 1 and hh == 1:
                    act(h1[:, 256 * mt:256 * (mt + 1)], ps[:, :256], AF.Relu,
                        bias=b1_sb[:, mt:mt + 1], scale=1.0)
                else:
                    dve.tensor_scalar(out=h1[:, 256 * mt:256 * (mt + 1)],
                                      in0=ps[:, :256],
                                      scalar1=b1_sb[:, mt:mt + 1],
                                      scalar2=0.0, op0=OP.add, op1=OP.max)

            for mt in range(16):
                units.append(lambda mt=mt: fc1_unit(mt))

            z2 = [z_pool.tile([128, 256], F32, tag=f"y{ot}", name=f"y{ot}")
                  for ot in range(4)]

            def fc2_unit(ot):
                ps = pch.tile([128, 512], F32, tag="ps", name="ps")
                for mt in range(16):
                    mm(ps[:, :256], w2_sb[mt][:, 128 * ot:128 * (ot + 1)],
                       h1[:, 256 * mt:256 * (mt + 1)], start=(mt == 0),
                       stop=(mt == 15))
                dve.scalar_tensor_tensor(out=z2[ot], in0=ps[:, :256],
                                         scalar=b2_sb[:, ot:ot + 1],
                                         in1=src1_bv[:, ot, c0:c0 + 256],
                                         op0=OP.add, op1=OP.add)

            def w2(ct, t2, g_sb, b_sb):
                o = o_pool.tile([128, 256], F32, tag="out", name="out")
                dve.tensor_scalar(out=o, in0=t2,
                                  scalar1=g_sb[:, ct:ct + 1],
                                  scalar2=b_sb[:, ct:ct + 1],
                                  op0=OP.mult, op1=OP.add)
                dma(out_d[128 * ct:128 * (ct + 1), c0:c0 + 256], o)

            for ot in range(4):
                units.append(lambda ot=ot: fc2_unit(ot))
            units.append(lambda: layernorm(z2, g2_sb, bt2_sb, w2))

        # ---- schedule ----
        import os
        STAGE = int(os.environ.get("KSTAGE", "9"))
        proj(wk8v, srcu8v, sup, k_sb[0], 0, bk_sb, 1.0 / SW)
        proj(wq8v, src8v, S, q_sb[0], 0, bq_sb, 1.0 / (SWQ / SQ))
        rearrange_qk(0)
        rearrange_qk(1)
        proj_v()
        proj(wk8v, srcu8v, sup, k_sb[1], 1, bk_sb, 1.0 / SW)
        proj(wq8v, src8v, S, q_sb[1], 1, bq_sb, 1.0 / (SWQ / SQ))
        rearrange_qk(2)
        rearrange_qk(3)
        load_chain_weights()

        NCH = int(os.environ.get("KCH", "4"))
        DRV = int(os.environ.get("KDRV", "1"))
        if STAGE >= 2:
            attention(0, 0)
        if STAGE >= 4 and NCH >= 1:
            chain_half(0, 0)
        if STAGE >= 3:
            attention(1, DRV)
        if STAGE >= 5 and NCH >= 2:
            chain_half(0, 1)
        if STAGE >= 3:
            attention(2, DRV)
        if STAGE >= 5 and NCH >= 3:
            chain_half(1, 0)
        if STAGE >= 3:
            attention(3, DRV * LASTD)
        if STAGE >= 5 and NCH >= 4:
            chain_half(1, 1)
        drain_units()
        if STAGE < 5:
            dbgt = o_pool.tile([128, 512], F32, tag="out", name="dbg")
            dve.tensor_copy(out=dbgt, in_=xnp[0][:, 0:512])
            for ct in range(4):
                dma(out_d[128 * ct:128 * (ct + 1), 0:512], dbgt)

    nc.compile()
    return nc


def _prep_core(c, src, idxs, sup, w):
    b, hg = c // 2, c % 2
    heads = list(range(HPC * hg, HPC * hg + HPC))
    KT = sup // 128
    f32 = np.float32
    st = np.ascontiguousarray(src[b].T)                       # [512, 2048]
    idx = idxs[b]
    su = len(idx)
    srcu = np.zeros((HID, sup), f32)
    srcu[:, :su] = st[:, idx]

    def dsplit(x, ncols):
        # [512, n] -> [128, 2(ctp), 2(sub), n] fp8
        return np.ascontiguousarray(
            x.reshape(2, 2, 128, ncols).transpose(2, 0, 1, 3)
        ).astype(f8dt).reshape(128, 4 * ncols)

    wqe = np.concatenate([w["Wm"] @ w["Wq"][64 * h:64 * (h + 1), :]
                          for h in heads])                    # [256, 512]
    bqe = np.concatenate([w["Wm"] @ w["bq"][64 * h:64 * (h + 1)] + w["bm"]
                          for h in heads])
    wks = np.concatenate([w["Wk"][64 * h:64 * (h + 1), :] for h in heads])
    bks = np.concatenate([w["bk"][64 * h:64 * (h + 1)] for h in heads])
    wvs = np.concatenate([w["Wv"][64 * h:64 * (h + 1), :] for h in heads])
    bvs = np.concatenate([w["bv"][64 * h:64 * (h + 1)] for h in heads])

    # wo8: [p=(par,d), jg, sub, of] = 16*Wo[of, 64*(4jg+2sub+par)+d]
    wo = w["Wo"]                                              # [of, f_in]
    wo8 = np.empty((2, 64, 2, 2, HID), f32)
    for par in range(2):
        for jg in range(2):
            for sub in range(2):
                j = 4 * jg + 2 * sub + par
                wo8[par, :, jg, sub, :] = SW * wo[:, 64 * j:64 * (j + 1)].T
    wo8 = wo8.reshape(128, 2, 2, HID).reshape(128, 4 * HID).astype(f8dt)

    mb = np.full(sup, NEG_BIG, f32)
    mb[:su] = 0.0
    return {
        "src8": dsplit(st, S),
        "srcu8": dsplit(srcu, sup),
        "src_res": np.ascontiguousarray(st[:, R * hg:R * (hg + 1)])
                   + w["bo"][:, None],
        "wq8": dsplit(SWQ * wqe.T, 256),
        "wk8": dsplit(SW * wks.T, 256),
        "wv8": dsplit(SW * wvs.T, 256),
        "wo8": wo8,
        "w1": np.ascontiguousarray(w["W1"].T).astype(bfdt),
        "w2": np.ascontiguousarray(w["W2"].T).astype(bfdt),
        "bq": (SWQ * bqe).reshape(2, 128).astype(f32),
        "bk": (SW * bks).reshape(2, 128).astype(f32),
        "bv": bvs.astype(f32),
        "b1": w["b1"].reshape(16, 128).astype(f32),
        "b2": w["b2"].reshape(4, 128).astype(f32),
        "g1": w["ln1_g"].reshape(4, 128).astype(f32),
        "bt1": w["ln1_b"].reshape(4, 128).astype(f32),
        "g2": w["ln2_g"].reshape(4, 128).astype(f32),
        "bt2": w["ln2_b"].reshape(4, 128).astype(f32),
        "mb": mb.reshape(KT, 128),
        "ident": np.eye(128).astype(bfdt),
    }


def kernel(**inputs):
    global last_results
    w = {k: np.asarray(v, np.float32) for k, v in inputs.items()
         if k not in ("src", "src_mask")}
    src = np.asarray(inputs["src"], np.float32)
    mask = np.asarray(inputs["src_mask"]).reshape(B, S)
    idxs = [np.nonzero(mask[b] != 0)[0] for b in range(B)]
    sup = max(256, ((max(len(i) for i in idxs) + 127) // 128) * 128)

    if sup not in _built_cache:
        _built_cache[sup] = build_bass(sup)
    nc = _built_cache[sup]

    in_maps = [_prep_core(c, src, idxs, sup, w) for c in range(N_CORES)]
    res = bass_utils.run_bass_kernel_spmd(nc, in_maps, core_ids=list(range(N_CORES)),
                                          **run_kwargs)
    last_results = res
    out = np.empty((B, S, HID), np.float32)
    for c in range(N_CORES):
        b, hg = c // 2, c % 2
        out[b, R * hg:R * (hg + 1), :] = res.results[c]["out_t"].T
    return out

